# revision 2
# baseline (speedup 1.0000x reference)
"""Self-contained Trainium2 kernel for nn_Attention_56607668961538 (v4).

kernel(**inputs) takes the FULL unsharded inputs (B=16, N=1024, C=1024),
shards data-parallel over batch across 8 NeuronCores (B_local=2 each),
runs a Bass/Tile attention kernel per core via run_bass_kernel_spmd, and
gathers the full output.

All bf16 (fp8/DoubleRow rejected: every config fails the 2e-2 gate in
numeric simulation; the bf16 baseline itself sits at 7.9e-3).  The
attention segment is ACT-exp paced (~352 fixed cycles per activation
instruction), so exps stay at the maximal [128,1024] width from PSUM.

v4 vs the original baseline:
  - rms/attn scale factors folded into the q rows of qkv_w (rmsnorm is
    scale invariant); the ssq mask carries 1/c^2 so the reduction
    un-scales exactly.  One plain selector constant then serves the
    q-scale, k-scale and 1/den broadcasts.
  - proj weights resident in SBUF, loaded once per rep instead of per
    batch (kills proj-weight prefetch stalls in the tail).
  - 1/den chain emitted at the end of attention so its DVE ops complete
    before the next phase1 floods the DVE queue.
  - V-tile PSUM evictions moved to the ACT engine (idle during phase1),
    so the DVE queue can keep up with the scale muls.
  - dummy 1-element Sqrt/Exp activations pre-warm the ACT table sets off
    the critical path; scale broadcasts deferred past the first V chunk
    so the sqrt->recip chain never stalls the PE.
  - wqk[0] DMA'd first so the opening QKV chain starts ASAP.
"""

import sys

sys.path.insert(0, "/opt/trn_rl_repo")

import numpy as np

from contextlib import ExitStack

import concourse.bass as bass
import concourse.mybir as mybir
import concourse.tile as tile

F32 = mybir.dt.float32
BF16 = mybir.dt.bfloat16
EPS = 1e-6


def build_attention(nc, B_local, N, C, H, reps=1):
    AF = mybir.ActivationFunctionType
    Dh = C // H             # 64
    assert Dh == 64
    KT = C // 128           # 8 contraction k-tiles
    NT = N // 128           # 8 token m-tiles
    TCH = 512               # token chunk (PSUM free-dim limit)
    NCH = N // TCH          # 2
    FQK = 2 * C // 128      # 16 q+k feature tiles
    VW = 512                # v-weight chunk width
    NVC = C // VW           # 2
    E = Dh + 1              # 65: Dh v-features + ones column (denominator)

    # ---- external I/O ----
    xT = nc.dram_tensor("xT", [B_local, C, N], BF16, kind="ExternalInput").ap()
    qk_wT = nc.dram_tensor("qk_wT", [FQK, 128, C], BF16,
                           kind="ExternalInput").ap()
    v_wT = nc.dram_tensor("v_wT", [C, C], BF16, kind="ExternalInput").ap()
    proj_wT = nc.dram_tensor("proj_wT", [KT, 128, C], BF16,
                             kind="ExternalInput").ap()
    proj_b = nc.dram_tensor("proj_b", [C], F32, kind="ExternalInput").ap()
    maskq = nc.dram_tensor("maskq", [C, H], BF16,
                           kind="ExternalInput").ap()
    maskk = nc.dram_tensor("maskk", [C, H], BF16,
                           kind="ExternalInput").ap()
    sel = nc.dram_tensor("sel", [H, C], BF16, kind="ExternalInput").ap()
    yT = nc.dram_tensor("yT", [B_local, C, N], F32, kind="ExternalOutput").ap()

    with tile.TileContext(nc) as tc, ExitStack() as ctx:
        singles = ctx.enter_context(tc.tile_pool(name="singles", bufs=1))
        wqkp = ctx.enter_context(tc.tile_pool(name="wqkp", bufs=1))
        pwp = ctx.enter_context(tc.tile_pool(name="pwp", bufs=1))
        vwp = ctx.enter_context(tc.tile_pool(name="vwp", bufs=2))
        xp = ctx.enter_context(tc.tile_pool(name="xp", bufs=2))
        qkp = ctx.enter_context(tc.tile_pool(name="qkp", bufs=21))
        sqp = ctx.enter_context(tc.tile_pool(name="sqp", bufs=2))
        vap = ctx.enter_context(tc.tile_pool(name="vap", bufs=8))
        ptp = ctx.enter_context(tc.tile_pool(name="ptp", bufs=3))
        stgp = ctx.enter_context(tc.tile_pool(name="stgp", bufs=2))
        aop = ctx.enter_context(tc.tile_pool(name="aop", bufs=8))
        atp = ctx.enter_context(tc.tile_pool(name="atp", bufs=8))
        denp = ctx.enter_context(tc.tile_pool(name="denp", bufs=1))
        ystp = ctx.enter_context(tc.tile_pool(name="ystp", bufs=2))

        # PSUM (8 banks): mmps 2x[128,512] (2) + stps 2x[128,1024] (4) +
        # avps 1x[65,1024] (2).  Wide [128,N] score tiles keep the exp ACT
        # instruction count minimal -- ACT pays ~352 fixed cycles per
        # instruction, and exp paces the whole attention segment.
        mmps = ctx.enter_context(tc.tile_pool(name="mmps", bufs=2,
                                              space="PSUM"))
        stps = ctx.enter_context(tc.tile_pool(name="stps", bufs=2,
                                              space="PSUM"))
        avps = ctx.enter_context(tc.tile_pool(name="avps", bufs=1,
                                              space="PSUM"))

        loop = ctx.enter_context(tc.For_i(0, reps, 1)) if reps > 1 else None

        # ---- constants / weights (once per rep) ----
        maskq_sb = singles.tile([128, KT, H], BF16)
        nc.sync.dma_start(out=maskq_sb,
                          in_=maskq.rearrange("(k p) h -> p k h", p=128))
        maskk_sb = singles.tile([128, KT, H], BF16)
        nc.sync.dma_start(out=maskk_sb,
                          in_=maskk.rearrange("(k p) h -> p k h", p=128))
        eps_sb = singles.tile([H, 1], F32)
        nc.vector.memset(eps_sb, EPS)
        sel_sb = singles.tile([H, C], BF16)
        nc.sync.dma_start(out=sel_sb, in_=sel)

        # lead-in: wqk[0] first on sync so the first QKV chain's weights
        # arrive before the remaining 15 weight tiles queue up
        wqk = [None] * FQK
        wqk[0] = wqkp.tile([128, KT, 128], BF16, name="wqk0")
        nc.sync.dma_start(out=wqk[0],
                          in_=qk_wT[0].rearrange("p (k f) -> p k f", f=128))
        x_pre = {}
        for tcn in range(NCH):
            xt0 = xp.tile([128, KT, TCH], BF16, tag="x", name=f"x0_{tcn}")
            nc.gpsimd.dma_start(
                out=xt0,
                in_=xT[0, :, tcn * TCH:(tcn + 1) * TCH].rearrange(
                    "(k p) t -> p k t", p=128))
            x_pre[tcn] = xt0
        for ft in range(1, FQK):
            wt_r = wqkp.tile([128, KT, 128], BF16, name=f"wqk{ft}")
            nc.sync.dma_start(
                out=wt_r, in_=qk_wT[ft].rearrange("p (k f) -> p k f", f=128))
            wqk[ft] = wt_r

        bias_sb = singles.tile([128, KT], F32)
        nc.sync.dma_start(out=bias_sb, in_=proj_b.rearrange("(k p) -> p k",
                                                            p=128))
        # proj weights: resident, loaded once per rep
        pw = []
        for mt in range(KT):
            pwt = pwp.tile([128, KT, 128], BF16, name=f"pw{mt}")
            nc.sync.dma_start(
                out=pwt, in_=proj_wT[mt].rearrange("p (k f) -> p k f", f=128))
            pw.append(pwt)

        state = {}

        def emit_phase1(b, fast=False, x_pre=None):
            qk = {}
            va = {}
            x_sb = {}
            # dummy 1-element Sqrt: pulls the ACT table load for the Sqrt
            # set into the qk-chain window instead of the critical chain
            warm = denp.tile([1, 1], F32, tag="warm", bufs=1,
                             name=f"warm{b}")
            nc.scalar.activation(out=warm, in_=eps_sb[0:1, :], func=AF.Sqrt)
            # ssq_all layout: [H, g*N + tcn*TCH] for g in {q,k}, tcn halves
            ssq_all = denp.tile([H, 2 * N], F32, tag="ssqa", bufs=1,
                                name=f"ssqa{b}")
            # pending ssq matmul, software-pipelined by one tile so the PE
            # never waits on the DVE square
            pend = []
            chain = {"ps": None, "cnt": 0, "off": None}

            seen = set()

            def flush_pend():
                # fast mode (attn pools idle): chain the 8 fi mask-matmuls
                # of a (g,tcn) group into one borrowed avps PSUM tile.
                # slow mode: single mask matmul into mmps + DVE add.
                if not pend:
                    return
                sq, fi, g, off = pend.pop()
                mask = maskq_sb if g == 0 else maskk_sb
                if fast:
                    if chain["cnt"] == 0:
                        chain["ps"] = avps.tile([H, TCH], F32, tag="av",
                                                name=f"ssqc{b}_{off}")
                    nc.tensor.matmul(chain["ps"], mask[:, fi], sq,
                                     start=(chain["cnt"] == 0),
                                     stop=(chain["cnt"] == KT - 1))
                    chain["cnt"] += 1
                    if chain["cnt"] == KT:
                        nc.vector.tensor_copy(ssq_all[:, off:off + TCH],
                                              chain["ps"])
                        chain["ps"] = None
                        chain["cnt"] = 0
                else:
                    ps2 = mmps.tile([128, TCH], F32, tag="mm")
                    nc.tensor.matmul(ps2[:H], mask[:, fi], sq,
                                     start=True, stop=True)
                    dst = ssq_all[:, off:off + TCH]
                    if off not in seen:
                        seen.add(off)
                        nc.vector.tensor_copy(dst, ps2[:H])
                    else:
                        nc.vector.tensor_add(dst, dst, ps2[:H])

            # g-outer so only one ssq PSUM chain is alive at a time
            for tcn in range(NCH):
                if x_pre is not None:
                    xt = x_pre[tcn]
                else:
                    xt = xp.tile([128, KT, TCH], BF16, tag="x",
                                 name=f"x{b}_{tcn}")
                    nc.gpsimd.dma_start(
                        out=xt,
                        in_=xT[b, :, tcn * TCH:(tcn + 1) * TCH].rearrange(
                            "(k p) t -> p k t", p=128))
                x_sb[tcn] = xt
                tsl = slice(tcn * TCH, (tcn + 1) * TCH)
                for g in range(2):
                    for fi in range(KT):
                        ft = g * KT + fi
                        if tcn == 0:
                            qk[ft] = qkp.tile([128, N], BF16, tag="qk",
                                              name=f"qk_{b}_{ft}")
                        mpool, mtag = (stps, "st") if fast else (mmps, "mm")
                        ps = mpool.tile([128, TCH], F32, tag=mtag)
                        for k in range(KT):
                            nc.tensor.matmul(ps, wqk[ft][:, k], xt[:, k],
                                             start=(k == 0),
                                             stop=(k == KT - 1))
                        nc.vector.tensor_copy(qk[ft][:, tsl], ps)
                        sq = sqp.tile([128, TCH], BF16, tag="sq")
                        nc.vector.tensor_mul(sq, qk[ft][:, tsl],
                                             qk[ft][:, tsl])
                        flush_pend()
                        pend.append((sq, fi, g, g * N + tcn * TCH))
            flush_pend()

            # one Sqrt ACT + one fast reciprocal per batch, then bf16 round
            nc.scalar.activation(out=ssq_all, in_=ssq_all, func=AF.Sqrt,
                                 bias=eps_sb, scale=1.0 / Dh)
            rtmp = denp.tile([H, 2 * N], F32, tag="scr", bufs=1,
                             name=f"rtmp{b}")
            nc.vector.reciprocal_approx_fast(out=rtmp, in_=ssq_all)
            invr_all = denp.tile([H, 2 * N], BF16, tag="invr", bufs=1,
                                 name=f"invr{b}")
            nc.vector.tensor_copy(invr_all, rtmp)

            # scale pass: PE selector matmul broadcasts the per-head inverse
            # rms to the 64 partitions of each head (fed just-in-time from
            # PSUM, no DMA latency), then one in-place DVE mul per tile.
            # Interleaved with the V matmuls to keep the PE stream dense;
            # ordered f-ascending / q,k adjacent to match attn consumption.
            scale_steps = [(g * KT + fi, half) for fi in range(KT)
                           for g in range(2) for half in range(NCH)]
            si = 0

            def emit_scale(n):
                nonlocal si
                for _ in range(n):
                    if si >= len(scale_steps):
                        return
                    ft, half = scale_steps[si]
                    si += 1
                    g, fi = divmod(ft, KT)
                    iv = invr_all[:, g * N + half * TCH:
                                  g * N + (half + 1) * TCH]
                    bs = mmps.tile([128, TCH], F32, tag="mm")
                    nc.tensor.matmul(bs, sel_sb[:, fi * 128:(fi + 1) * 128],
                                     iv, start=True, stop=True)
                    hs = slice(half * TCH, (half + 1) * TCH)
                    nc.vector.tensor_mul(qk[ft][:, hs], qk[ft][:, hs], bs)

            for vc in range(NVC):
                vwt = vwp.tile([128, KT, VW], BF16, tag="vw")
                nc.gpsimd.dma_start(
                    out=vwt,
                    in_=v_wT[:, vc * VW:(vc + 1) * VW].rearrange(
                        "(k p) f -> p k f", p=128))
                for j in range(NT):
                    if vc == 0:
                        va[j] = vap.tile([128, H, E], BF16, tag="va",
                                         name=f"va_{b}_{j}")
                        nc.vector.memset(va[j][:, :, Dh:E], 1.0)
                    mpool, mtag = (stps, "st") if fast else (mmps, "mm")
                    ps = mpool.tile([128, TCH], F32, tag=mtag)
                    xsrc = x_sb[j // (TCH // 128)]
                    tm = j % (TCH // 128)
                    for k in range(KT):
                        nc.tensor.matmul(
                            ps[:, :VW], xsrc[:, k, tm * 128:(tm + 1) * 128],
                            vwt[:, k], start=(k == 0), stop=(k == KT - 1))
                    # ACT evict: keeps the DVE queue free for the scale
                    # muls so the mmps buffer rotation never stalls the PE
                    nc.scalar.copy(
                        va[j][:, vc * (VW // Dh):(vc + 1) * (VW // Dh), 0:Dh],
                        ps[:, :VW].rearrange("p (h e) -> p h e", e=Dh))
                    # no scale steps during vc=0: the first 8 V chains cover
                    # the Sqrt->recip->copy latency so bs matmuls never stall
                    if vc > 0:
                        emit_scale(4)
            emit_scale(len(scale_steps))
            # dummy 1-element Exp: reloads the Exp table set while the PE
            # finishes phase1, so attn's first real exp doesn't pay it
            warm2 = denp.tile([1, 1], F32, tag="warm2", bufs=1,
                              name=f"warm2_{b}")
            nc.scalar.activation(out=warm2, in_=eps_sb[0:1, :], func=AF.Exp)
            state[b] = dict(qk=qk, va=va)

        def emit_attn(b):
            qk = state[b]["qk"]
            va = state[b]["va"]
            den = denp.tile([H, N], BF16, tag="den", bufs=2, name=f"den{b}")
            ao_all = {}
            for f in range(KT):
                qt = qk[f]
                kt = qk[KT + f]
                ao = aop.tile([128, N], BF16, tag="ao", name=f"attn_{b}_{f}")
                for h2 in range(2):
                    h = 2 * f + h2
                    psl = slice(h2 * Dh, (h2 + 1) * Dh)
                    # one wide [128,N] score tile + ONE exp ACT per j: exp
                    # paces this whole segment and pays ~352 fixed cycles
                    # per instruction, so widest-possible exps win
                    av = avps.tile([E, N], F32, tag="av",
                                   name=f"av_{b}_{h}")
                    for j in range(NT):
                        ksl = slice(j * 128, (j + 1) * 128)
                        st = stps.tile([128, N], F32, tag="st")
                        for half in range(NCH):
                            hs = slice(half * TCH, (half + 1) * TCH)
                            nc.tensor.matmul(st[:, hs], kt[psl, ksl],
                                             qt[psl, hs],
                                             start=True, stop=True)
                        pt = ptp.tile([128, N], BF16, tag="pt")
                        nc.scalar.activation(out=pt, in_=st, func=AF.Exp)
                        for half in range(NCH):
                            hs = slice(half * TCH, (half + 1) * TCH)
                            nc.tensor.matmul(av[:, hs], va[j][:, h, :],
                                             pt[:, hs], start=(j == 0),
                                             stop=(j == NT - 1))
                    stg = stgp.tile([E, N], BF16, tag="stage")
                    nc.vector.tensor_copy(stg, av)
                    nc.sync.dma_start(out=ao[psl, :], in_=stg[0:Dh, :])
                    nc.sync.dma_start(out=den[h:h + 1, :], in_=stg[Dh:E, :])
                ao_all[f] = ao
            # invden chain emitted here (not in emit_proj) so its DVE ops
            # queue ahead of the next phase1's DVE work and finish early
            dscr = denp.tile([H, 2 * N], F32, tag="scr", bufs=1,
                             name=f"dscr{b}")
            nc.vector.tensor_copy(dscr[:, 0:N], den)
            nc.vector.reciprocal_approx_fast(out=dscr[:, N:2 * N],
                                             in_=dscr[:, 0:N])
            invden = denp.tile([H, N], BF16, tag="invd", bufs=1,
                               name=f"invd{b}")
            nc.vector.tensor_copy(invden, dscr[:, N:2 * N])
            state[b]["ao"] = ao_all
            state[b]["invden"] = invden
            state[b]["qk"] = None
            state[b]["va"] = None

        def emit_proj(b, fast=False):
            ao = state[b]["ao"]
            invden = state[b]["invden"]

            for half in range(NCH):
                hs = slice(half * TCH, (half + 1) * TCH)
                at = []
                for f in range(KT):
                    bd = mmps.tile([128, TCH], F32, tag="mm")
                    nc.tensor.matmul(bd, sel_sb[:, f * 128:(f + 1) * 128],
                                     invden[:, hs], start=True, stop=True)
                    a = atp.tile([128, TCH], BF16, tag="at")
                    nc.vector.tensor_mul(a, ao[f][:, hs], bd)
                    at.append(a)
                for mt in range(KT):
                    mpool, mtag = (stps, "st") if fast else (mmps, "mm")
                    ps = mpool.tile([128, TCH], F32, tag=mtag)
                    for k in range(KT):
                        nc.tensor.matmul(ps, pw[mt][:, k], at[k],
                                         start=(k == 0), stop=(k == KT - 1))
                    yst = ystp.tile([128, TCH], F32, tag="yst")
                    nc.scalar.add(yst, ps, bias_sb[:, mt:mt + 1])
                    nc.sync.dma_start(out=yT[b, mt * 128:(mt + 1) * 128, hs],
                                      in_=yst)
                del at

        emit_phase1(0, fast=True, x_pre=x_pre)
        emit_attn(0)
        for b in range(1, B_local):
            emit_phase1(b)
            emit_proj(b - 1)
            emit_attn(b)
        emit_proj(B_local - 1, fast=True)

    return nc


def prep_inputs(x, qkv_w, proj_w, proj_b, q_norm_w, k_norm_w, n_cores):
    """Host-side prep: shard over batch, pre-transpose, cast to bf16, fold
    the rms/attn scales into the q rows of qkv_w, build the replicated
    block-diagonal ssq masks."""
    import ml_dtypes

    bf16 = ml_dtypes.bfloat16
    B, N, C = x.shape
    H = C // 64
    Dh = 64
    B_local = B // n_cores
    scale = Dh ** -0.5

    # per-dim fold: c[d] = scale * q_norm_w[d] * k_norm_w[d] onto q rows
    c = (scale * np.asarray(q_norm_w, np.float64)
         * np.asarray(k_norm_w, np.float64)).astype(np.float64)
    qkv_w = np.asarray(qkv_w, np.float64).copy()
    rows = np.arange(C)
    qkv_w[:C] *= c[rows % Dh][:, None]

    qkv_wT = np.ascontiguousarray(qkv_w.T)          # [C, 3C]
    qk_wT = np.ascontiguousarray(
        qkv_wT[:, :2 * C].reshape(C // 128, 128, 2 * C // 128, 128)
        .transpose(2, 1, 0, 3).reshape(2 * C // 128, 128, C)).astype(bf16)
    v_wT = np.ascontiguousarray(qkv_wT[:, 2 * C:]).astype(bf16)
    proj_wT = np.ascontiguousarray(
        np.asarray(proj_w, np.float64).T.reshape(C // 128, 128, C // 128, 128)
        .transpose(2, 1, 0, 3).reshape(C // 128, 128, C)).astype(bf16)

    # compact ssq masks [C, H]: feature -> its head; q side weighted 1/c^2
    # to exactly un-scale the fold of c into the q rows of qkv_w
    heads = np.arange(C) // Dh
    match = (heads[:, None] == np.arange(H)[None, :])
    invc2 = 1.0 / (c * c)
    maskq_np = (match * invc2[np.arange(C) % Dh][:, None]).astype(bf16)
    maskk_np = match.astype(bf16)
    sel_np = np.ascontiguousarray(match.T).astype(bf16)   # [H, C] selector

    shared = dict(qk_wT=qk_wT, v_wT=v_wT, proj_wT=proj_wT,
                  proj_b=np.asarray(proj_b, np.float32),
                  maskq=maskq_np, maskk=maskk_np, sel=sel_np)
    in_maps = []
    for i in range(n_cores):
        xs = x[i * B_local:(i + 1) * B_local]        # [B_local, N, C]
        xTl = np.ascontiguousarray(xs.transpose(0, 2, 1)).astype(bf16)
        in_maps.append(dict(xT=xTl, **shared))
    return in_maps, dict(B=B, N=N, C=C, H=H, B_local=B_local)


def gather_output(results, meta):
    B, N, C, B_local = meta["B"], meta["N"], meta["C"], meta["B_local"]
    y = np.empty((B, N, C), np.float32)
    for i, r in enumerate(results):
        yTl = r["yT"]                                # [B_local, C, N]
        y[i * B_local:(i + 1) * B_local] = yTl.transpose(0, 2, 1)
    return y


N_CORES = 8
_CACHE = {}


def _get_nc():
    if "nc" not in _CACHE:
        from concourse import bacc

        nc = bacc.Bacc("TRN2", target_bir_lowering=False, debug=False,
                       num_devices=N_CORES)
        build_attention(nc, B_local=16 // N_CORES, N=1024, C=1024, H=16)
        nc.compile()
        _CACHE["nc"] = nc
    return _CACHE["nc"]


def run_sharded(in_maps, trace=False):
    from concourse.bass_utils import run_bass_kernel_spmd

    return run_bass_kernel_spmd(_get_nc(), in_maps,
                                core_ids=list(range(N_CORES)), trace=trace)


def kernel(x, qkv_w, proj_w, proj_b, q_norm_w, k_norm_w):
    in_maps, meta = prep_inputs(np.asarray(x), np.asarray(qkv_w),
                                np.asarray(proj_w), np.asarray(proj_b),
                                np.asarray(q_norm_w), np.asarray(k_norm_w),
                                N_CORES)
    res = run_sharded(in_maps)
    return gather_output(res.results, meta)
